# revision 2
# baseline (speedup 1.0000x reference)
"""Trainium2 Bass kernel for nn_Disease_Guide_ROI (dense_transformer).

Math notes (verified vs reference to ~4e-7 rel err):
  - softmax over a length-1 axis is exactly 1.0, so the attention block
    collapses to x1 = v * weight; q/k/cls_out/cls_w/cls_b are dead.
  - only the v half of the kv projection is needed.
  - the GRU update after iteration 3 is dead (weight3 unused).
Pipeline per sample (C=90):
  v  = kv_w[C:2C] @ x + kv_b[C:2C]
  w  = w0;  x1 = v*w;  w = GRU(x1, w)   (twice)
  x1 = v*w;  out = proj_w @ x1 + proj_b

Device layout: channel-major [90, N] tiles; host pre-transposes x to (90, B)
and post-transposes the (90, B) output, so all DMAs are contiguous 2KB rows.
Sharding: pure data parallel over 8 cores (B/8 = 16384 samples each).
"""

import sys

if "/opt/trn_rl_repo" not in sys.path:
    sys.path.insert(0, "/opt/trn_rl_repo")

import numpy as np
from contextlib import ExitStack

B = 131072
C = 90
NCORES = 8
BC = B // NCORES  # 16384
CHUNK = 512
NCHUNK = BC // CHUNK  # 32

# column indices into the per-partition constant vector tensor
(CV_BV, CV_W0, CV_HN1, CV_BR1, CV_BZ1, CV_BIHN, CV_BR2, CV_BZ2, CV_BHHN,
 CV_BP) = range(10)
NCV = 10

_BUILD_CACHE = {}


def _build_nc():
    import concourse.bacc as bacc
    import concourse.tile as tile
    import concourse.mybir as mybir

    f32 = mybir.dt.float32
    Alu = mybir.AluOpType
    Act = mybir.ActivationFunctionType

    nc = bacc.Bacc(None, target_bir_lowering=False)
    with ExitStack() as ctx:
        tc = ctx.enter_context(tile.TileContext(nc))
        xT = nc.dram_tensor("xT", [C, BC], f32, kind="ExternalInput")
        wmat = nc.dram_tensor("wmat", [C, 8 * C], f32, kind="ExternalInput")
        cvec = nc.dram_tensor("cvec", [C, NCV], f32, kind="ExternalInput")
        outT = nc.dram_tensor("outT", [C, BC], f32, kind="ExternalOutput")

        const = ctx.enter_context(tc.tile_pool(name="const", bufs=1))
        io = ctx.enter_context(tc.tile_pool(name="io", bufs=4))
        work = ctx.enter_context(tc.tile_pool(name="work", bufs=4))
        ps = ctx.enter_context(tc.tile_pool(name="ps", bufs=2, space="PSUM"))

        Wm = const.tile([C, 8 * C], f32)
        nc.sync.dma_start(out=Wm, in_=wmat[:, :])
        cv = const.tile([C, NCV], f32)
        nc.sync.dma_start(out=cv, in_=cvec[:, :])

        kvT = Wm[:, 0 * C:1 * C]
        wihT_r = Wm[:, 1 * C:2 * C]
        wihT_z = Wm[:, 2 * C:3 * C]
        wihT_n = Wm[:, 3 * C:4 * C]
        whhT_r = Wm[:, 4 * C:5 * C]
        whhT_z = Wm[:, 5 * C:6 * C]
        whhT_n = Wm[:, 6 * C:7 * C]
        projT = Wm[:, 7 * C:8 * C]

        def col(i):
            return cv[:, i:i + 1]

        for ch in range(NCHUNK):
            sl = slice(ch * CHUNK, (ch + 1) * CHUNK)

            x_cm = io.tile([C, CHUNK], f32, tag="x")
            nc.sync.dma_start(out=x_cm, in_=xT[:, sl])

            pv = ps.tile([C, CHUNK], f32, tag="pv", bufs=2)
            nc.tensor.matmul(pv, kvT, x_cm, start=True, stop=True)
            v = work.tile([C, CHUNK], f32, tag="v", bufs=3)
            nc.scalar.activation(v, pv, Act.Identity, bias=col(CV_BV))

            # ---- iteration 1 (hidden = w0 per-channel constant) ----
            x1 = work.tile([C, CHUNK], f32, tag="x1", bufs=4)
            nc.vector.tensor_scalar_mul(x1, v, col(CV_W0))

            pr = ps.tile([C, CHUNK], f32, tag="gate", bufs=4)
            nc.tensor.matmul(pr, wihT_r, x1, start=True, stop=True)
            pz = ps.tile([C, CHUNK], f32, tag="gate", bufs=4)
            nc.tensor.matmul(pz, wihT_z, x1, start=True, stop=True)
            pi = ps.tile([C, CHUNK], f32, tag="gate", bufs=4)
            nc.tensor.matmul(pi, wihT_n, x1, start=True, stop=True)

            r = work.tile([C, CHUNK], f32, tag="g", bufs=6)
            nc.scalar.activation(r, pr, Act.Sigmoid, bias=col(CV_BR1))
            z = work.tile([C, CHUNK], f32, tag="g", bufs=6)
            nc.scalar.activation(z, pz, Act.Sigmoid, bias=col(CV_BZ1))
            t2 = work.tile([C, CHUNK], f32, tag="g", bufs=6)
            # (r * hn1c) + i_n
            nc.vector.scalar_tensor_tensor(
                t2, r, col(CV_HN1), pi, Alu.mult, Alu.add)
            n = work.tile([C, CHUNK], f32, tag="g", bufs=6)
            nc.scalar.activation(n, t2, Act.Tanh, bias=col(CV_BIHN))

            # w1 = n - z*(n - w0)
            u = work.tile([C, CHUNK], f32, tag="g", bufs=6)
            nc.vector.tensor_scalar(u, n, col(CV_W0), None, Alu.subtract)
            m = work.tile([C, CHUNK], f32, tag="g", bufs=6)
            nc.vector.tensor_tensor(m, z, u, Alu.mult)
            w1 = work.tile([C, CHUNK], f32, tag="w", bufs=3)
            nc.vector.tensor_tensor(w1, n, m, Alu.subtract)

            # ---- iteration 2 ----
            x1b = work.tile([C, CHUNK], f32, tag="x1", bufs=4)
            nc.vector.tensor_tensor(x1b, v, w1, Alu.mult)

            pr2 = ps.tile([C, CHUNK], f32, tag="gate", bufs=4)
            nc.tensor.matmul(pr2, wihT_r, x1b, start=True, stop=False)
            nc.tensor.matmul(pr2, whhT_r, w1, start=False, stop=True)
            pz2 = ps.tile([C, CHUNK], f32, tag="gate", bufs=4)
            nc.tensor.matmul(pz2, wihT_z, x1b, start=True, stop=False)
            nc.tensor.matmul(pz2, whhT_z, w1, start=False, stop=True)
            pi2 = ps.tile([C, CHUNK], f32, tag="gate", bufs=4)
            nc.tensor.matmul(pi2, wihT_n, x1b, start=True, stop=True)
            ph2 = ps.tile([C, CHUNK], f32, tag="gate", bufs=4)
            nc.tensor.matmul(ph2, whhT_n, w1, start=True, stop=True)

            r2 = work.tile([C, CHUNK], f32, tag="g", bufs=6)
            nc.scalar.activation(r2, pr2, Act.Sigmoid, bias=col(CV_BR2))
            z2 = work.tile([C, CHUNK], f32, tag="g", bufs=6)
            nc.scalar.activation(z2, pz2, Act.Sigmoid, bias=col(CV_BZ2))
            t = work.tile([C, CHUNK], f32, tag="g", bufs=6)
            # (h_n + b_hh_n) * r2
            nc.vector.scalar_tensor_tensor(
                t, ph2, col(CV_BHHN), r2, Alu.add, Alu.mult)
            t2b = work.tile([C, CHUNK], f32, tag="g", bufs=6)
            nc.vector.tensor_tensor(t2b, t, pi2, Alu.add)
            n2 = work.tile([C, CHUNK], f32, tag="g", bufs=6)
            nc.scalar.activation(n2, t2b, Act.Tanh, bias=col(CV_BIHN))

            # w2 = n2 - z2*(n2 - w1); x1c = v * w2
            u2 = work.tile([C, CHUNK], f32, tag="g", bufs=6)
            nc.vector.tensor_tensor(u2, n2, w1, Alu.subtract)
            m2 = work.tile([C, CHUNK], f32, tag="g", bufs=6)
            nc.vector.tensor_tensor(m2, z2, u2, Alu.mult)
            w2 = work.tile([C, CHUNK], f32, tag="w", bufs=3)
            nc.vector.tensor_tensor(w2, n2, m2, Alu.subtract)
            x1c = work.tile([C, CHUNK], f32, tag="x1", bufs=4)
            nc.vector.tensor_tensor(x1c, v, w2, Alu.mult)

            # ---- output projection ----
            po = ps.tile([C, CHUNK], f32, tag="po", bufs=2)
            nc.tensor.matmul(po, projT, x1c, start=True, stop=True)
            o = io.tile([C, CHUNK], f32, tag="o")
            nc.scalar.activation(o, po, Act.Identity, bias=col(CV_BP))
            nc.sync.dma_start(out=outT[:, sl], in_=o)

    nc.compile()
    return nc


def _get_nc():
    if "nc" not in _BUILD_CACHE:
        _BUILD_CACHE["nc"] = _build_nc()
    return _BUILD_CACHE["nc"]


def _prep_consts(w0, kv_w, kv_b, w_ih, w_hh, b_ih, b_hh, proj_w, proj_b):
    w0v = np.asarray(w0, np.float32).reshape(C)
    kv_w = np.asarray(kv_w, np.float32)
    kv_b = np.asarray(kv_b, np.float32)
    w_ih = np.asarray(w_ih, np.float32)
    w_hh = np.asarray(w_hh, np.float32)
    b_ih = np.asarray(b_ih, np.float32)
    b_hh = np.asarray(b_hh, np.float32)
    proj_w = np.asarray(proj_w, np.float32)
    proj_b = np.asarray(proj_b, np.float32)

    wmat = np.concatenate(
        [
            kv_w[C:2 * C].T,           # kvT
            w_ih[0:C].T,               # wihT_r
            w_ih[C:2 * C].T,           # wihT_z
            w_ih[2 * C:3 * C].T,       # wihT_n
            w_hh[0:C].T,               # whhT_r
            w_hh[C:2 * C].T,           # whhT_z
            w_hh[2 * C:3 * C].T,       # whhT_n
            proj_w.T,                  # projT
        ],
        axis=1,
    ).astype(np.float32)
    wmat = np.ascontiguousarray(wmat)

    gh1 = w0v @ w_hh.T + b_hh  # (270,) iter-1 hidden gate contribution
    cvec = np.zeros((C, NCV), np.float32)
    cvec[:, CV_BV] = kv_b[C:2 * C]
    cvec[:, CV_W0] = w0v
    cvec[:, CV_HN1] = gh1[2 * C:3 * C]
    cvec[:, CV_BR1] = b_ih[0:C] + gh1[0:C]
    cvec[:, CV_BZ1] = b_ih[C:2 * C] + gh1[C:2 * C]
    cvec[:, CV_BIHN] = b_ih[2 * C:3 * C]
    cvec[:, CV_BR2] = b_ih[0:C] + b_hh[0:C]
    cvec[:, CV_BZ2] = b_ih[C:2 * C] + b_hh[C:2 * C]
    cvec[:, CV_BHHN] = b_hh[2 * C:3 * C]
    cvec[:, CV_BP] = proj_b
    return wmat, cvec


def _run(inputs, trace=False):
    from concourse.bass_utils import run_bass_kernel_spmd

    x = np.asarray(inputs["x"], np.float32).reshape(B, C)
    wmat, cvec = _prep_consts(
        inputs["w0"], inputs["kv_w"], inputs["kv_b"], inputs["w_ih"],
        inputs["w_hh"], inputs["b_ih"], inputs["b_hh"], inputs["proj_w"],
        inputs["proj_b"])

    xT = np.ascontiguousarray(x.T)  # (C, B)
    in_maps = []
    for c in range(NCORES):
        in_maps.append({
            "xT": np.ascontiguousarray(xT[:, c * BC:(c + 1) * BC]),
            "wmat": wmat,
            "cvec": cvec,
        })

    nc = _get_nc()
    res = run_bass_kernel_spmd(
        nc, in_maps, core_ids=list(range(NCORES)), trace=trace)
    outT = np.concatenate([res.results[c]["outT"] for c in range(NCORES)],
                          axis=1)  # (C, B)
    out = np.ascontiguousarray(outT.T).astype(np.float32)  # (B, C)
    return out, res


def kernel(**inputs):
    out, _ = _run(inputs, trace=False)
    return out


# revision 5
# speedup vs baseline: 2.7767x; 2.7767x over previous
"""Trainium2 Bass kernel for nn_Disease_Guide_ROI (dense_transformer).

Math notes (verified vs reference, ~4e-7 rel err in fp32):
  - softmax over a length-1 axis is exactly 1.0 => attention collapses to
    x1 = v * weight; q/k/cls_out/cls_w/cls_b are dead.
  - only the v half of the kv projection is needed.
  - the GRU update after iteration 3 is dead (weight3 unused).
  - iteration-1 gates are affine in x (hidden = w0 const): host-compose
    W_g1 = w_ih_g @ diag(w0) @ kv_v so they come straight from x.
  - r1*hn1c is a diag-matmul (hn1c const) accumulated into the n-gate psum.
  - with zc = 1-z (sigmoid with scale=-1):  w1 = w0 + m',
    m' = zc1*(n1-w0);  x1c = x1b + v*m2, m2 = zc2*((n2-w0)-m'),
    and out = proj(x1b) + proj(m2*v) via PSUM accumulation.

Precision: fp32 elementwise; matmul operands in float32r (fp32 rounded to
11 mantissa bits, full-rate on the PE vs 4x-slower plain fp32).
Layout: channel-major [90, N]; host pre-transposes x to (90, B) and
post-transposes the (90, B) output so every DMA moves contiguous rows.
Sharding: pure data parallel, B/8 = 16384 samples per core.
"""

import sys

if "/opt/trn_rl_repo" not in sys.path:
    sys.path.insert(0, "/opt/trn_rl_repo")

import numpy as np
from contextlib import ExitStack

B = 131072
C = 90
NCORES = 8
BC = B // NCORES  # 16384
CHUNK = 1024
NCHUNK = BC // CHUNK  # 16
MMN = 512  # matmul moving free dim (one fp32 PSUM bank)

# column indices into the per-partition constant vector tensor
(CV_BV, CV_W0, CV_BR1, CV_NBZ1, CV_BN1, CV_BR2, CV_NBZ2, CV_BIHN, CV_BHHN,
 CV_BP) = range(10)
NCV = 10
NW = 12  # number of [90,90] lhsT matrices stacked in wmat

_BUILD_CACHE = {}


def _round_f32r(a):
    """Host equivalent of the device fp32 -> float32r rounding: round
    half-up to 11 mantissa bits, fp32 bit layout."""
    bits = np.ascontiguousarray(a, np.float32).view(np.uint32)
    out = ((bits + np.uint32(0x800)) & np.uint32(0xFFFFF000)).view(np.float32)
    return np.ascontiguousarray(out)


def _build_nc():
    import concourse.bacc as bacc
    import concourse.tile as tile
    import concourse.mybir as mybir

    f32 = mybir.dt.float32
    f32r = mybir.dt.float32r
    Alu = mybir.AluOpType
    Act = mybir.ActivationFunctionType

    nc = bacc.Bacc(None, target_bir_lowering=False)
    with ExitStack() as ctx:
        tc = ctx.enter_context(tile.TileContext(nc))
        xT = nc.dram_tensor("xT", [C, BC], f32r, kind="ExternalInput")
        wmat = nc.dram_tensor("wmat", [C, NW * C], f32r, kind="ExternalInput")
        cvec = nc.dram_tensor("cvec", [C, NCV], f32, kind="ExternalInput")
        outT = nc.dram_tensor("outT", [C, BC], f32, kind="ExternalOutput")

        const = ctx.enter_context(tc.tile_pool(name="const", bufs=1))
        io = ctx.enter_context(tc.tile_pool(name="io", bufs=3))
        work = ctx.enter_context(tc.tile_pool(name="work", bufs=3))
        ps = ctx.enter_context(tc.tile_pool(name="ps", bufs=3, space="PSUM"))

        Wm = const.tile([C, NW * C], f32r)
        nc.sync.dma_start(out=Wm, in_=wmat[:, :])
        cv = const.tile([C, NCV], f32)
        nc.sync.dma_start(out=cv, in_=cvec[:, :])

        (kvT, W1rT, W1zT, W1nT, diagH, wihT_r, wihT_z, wihT_n,
         whhT_r, whhT_z, whhT_n, projT) = (
            Wm[:, i * C:(i + 1) * C] for i in range(NW))

        def col(i):
            return cv[:, i:i + 1]

        def mm2(out_t, lhsT, rhs, start=True, stop=True):
            # one logical matmul over a [C, CHUNK] tile = 2 x N=512 matmuls
            for h in range(CHUNK // MMN):
                nc.tensor.matmul(out_t[:, h * MMN:(h + 1) * MMN], lhsT,
                                 rhs[:, h * MMN:(h + 1) * MMN],
                                 start=start, stop=stop)

        for ch in range(NCHUNK):
            sl = slice(ch * CHUNK, (ch + 1) * CHUNK)

            x_r = io.tile([C, CHUNK], f32r, tag="x")
            nc.sync.dma_start(out=x_r, in_=xT[:, sl])

            # ---- v projection ----
            pv = ps.tile([C, CHUNK], f32, tag="g")
            mm2(pv, kvT, x_r)
            v = work.tile([C, CHUNK], f32, tag="v")
            nc.scalar.activation(v, pv, Act.Identity, bias=col(CV_BV))

            # ---- iteration 1 (gates straight from x; hidden = w0) ----
            pr1 = ps.tile([C, CHUNK], f32, tag="g")
            mm2(pr1, W1rT, x_r)
            r1 = work.tile([C, CHUNK], f32r, tag="g1")
            nc.scalar.activation(r1, pr1, Act.Sigmoid, bias=col(CV_BR1))

            pz1 = ps.tile([C, CHUNK], f32, tag="g")
            mm2(pz1, W1zT, x_r)
            zc1 = work.tile([C, CHUNK], f32, tag="g1")
            nc.scalar.activation(zc1, pz1, Act.Sigmoid, bias=col(CV_NBZ1),
                                 scale=-1.0)

            pi1 = ps.tile([C, CHUNK], f32, tag="g")
            mm2(pi1, W1nT, x_r, start=True, stop=False)
            mm2(pi1, diagH, r1, start=False, stop=True)  # += r1 * hn1c
            n1 = work.tile([C, CHUNK], f32, tag="g1")
            nc.scalar.activation(n1, pi1, Act.Tanh, bias=col(CV_BN1))

            # m' = zc1*(n1 - w0);  w1 = w0 + m'
            mp = work.tile([C, CHUNK], f32, tag="g1")
            nc.vector.scalar_tensor_tensor(
                mp, n1, col(CV_W0), zc1, Alu.subtract, Alu.mult)
            w1 = work.tile([C, CHUNK], f32r, tag="w")
            nc.vector.tensor_scalar(w1, mp, col(CV_W0), None, Alu.add)
            x1b = work.tile([C, CHUNK], f32r, tag="x1")
            nc.vector.tensor_tensor(x1b, v, w1, Alu.mult)

            # ---- iteration 2 ----
            pr2 = ps.tile([C, CHUNK], f32, tag="g")
            mm2(pr2, wihT_r, x1b, start=True, stop=False)
            mm2(pr2, whhT_r, w1, start=False, stop=True)
            r2 = work.tile([C, CHUNK], f32, tag="g2")
            nc.scalar.activation(r2, pr2, Act.Sigmoid, bias=col(CV_BR2))

            pz2 = ps.tile([C, CHUNK], f32, tag="g")
            mm2(pz2, wihT_z, x1b, start=True, stop=False)
            mm2(pz2, whhT_z, w1, start=False, stop=True)
            zc2 = work.tile([C, CHUNK], f32, tag="g2")
            nc.scalar.activation(zc2, pz2, Act.Sigmoid, bias=col(CV_NBZ2),
                                 scale=-1.0)

            pi2 = ps.tile([C, CHUNK], f32, tag="g")
            mm2(pi2, wihT_n, x1b)
            ph2 = ps.tile([C, CHUNK], f32, tag="g")
            mm2(ph2, whhT_n, w1)

            t = work.tile([C, CHUNK], f32, tag="g2")
            nc.vector.scalar_tensor_tensor(
                t, ph2, col(CV_BHHN), r2, Alu.add, Alu.mult)
            t2b = work.tile([C, CHUNK], f32, tag="g2")
            nc.vector.tensor_tensor(t2b, t, pi2, Alu.add)
            n2 = work.tile([C, CHUNK], f32, tag="g2")
            nc.scalar.activation(n2, t2b, Act.Tanh, bias=col(CV_BIHN))

            # u2 = (n2 - w0) - m' = n2 - w1 ;  m2 = zc2*u2 ;  s = v*m2
            u2 = work.tile([C, CHUNK], f32, tag="g2")
            nc.vector.scalar_tensor_tensor(
                u2, n2, col(CV_W0), mp, Alu.subtract, Alu.subtract)
            m2 = work.tile([C, CHUNK], f32, tag="g2")
            nc.gpsimd.tensor_tensor(m2, zc2, u2, Alu.mult)
            s = work.tile([C, CHUNK], f32r, tag="x1")
            nc.vector.tensor_tensor(s, v, m2, Alu.mult)

            # ---- output projection: out = proj(x1b) + proj(s) + bp ----
            po = ps.tile([C, CHUNK], f32, tag="po", bufs=1)
            mm2(po, projT, x1b, start=True, stop=False)
            mm2(po, projT, s, start=False, stop=True)
            o = io.tile([C, CHUNK], f32, tag="o")
            nc.vector.tensor_scalar(o, po, col(CV_BP), None, Alu.add)
            nc.sync.dma_start(out=outT[:, sl], in_=o)

    nc.compile()
    return nc


def _get_nc():
    if "nc" not in _BUILD_CACHE:
        _BUILD_CACHE["nc"] = _build_nc()
    return _BUILD_CACHE["nc"]


def _prep_consts(w0, kv_w, kv_b, w_ih, w_hh, b_ih, b_hh, proj_w, proj_b):
    f8 = np.float64
    w0v = np.asarray(w0, f8).reshape(C)
    kv_w = np.asarray(kv_w, f8)
    kv_b = np.asarray(kv_b, f8)
    w_ih = np.asarray(w_ih, f8)
    w_hh = np.asarray(w_hh, f8)
    b_ih = np.asarray(b_ih, f8)
    b_hh = np.asarray(b_hh, f8)
    proj_w = np.asarray(proj_w, f8)
    proj_b = np.asarray(proj_b, f8)

    kv_v = kv_w[C:2 * C]
    b_v = kv_b[C:2 * C]
    gh1 = w0v @ w_hh.T + b_hh  # iter-1 hidden gate contribution (const)

    wg = {}
    for i, g in enumerate(("r", "z", "n")):
        wg[g] = (w_ih[i * C:(i + 1) * C] * w0v[None, :]) @ kv_v

    mats = [
        kv_v.T,                          # kvT
        wg["r"].T, wg["z"].T, wg["n"].T,  # W1gT (iter1 gates from x)
        np.diag(gh1[2 * C:3 * C]),       # diagH (r1 * hn1c)
        w_ih[0:C].T, w_ih[C:2 * C].T, w_ih[2 * C:3 * C].T,
        w_hh[0:C].T, w_hh[C:2 * C].T, w_hh[2 * C:3 * C].T,
        proj_w.T,
    ]
    wmat = _round_f32r(np.concatenate(mats, axis=1))

    bgate1 = {g: w_ih[i * C:(i + 1) * C] @ (w0v * b_v) + b_ih[i * C:(i + 1) * C]
              for i, g in enumerate(("r", "z", "n"))}
    cvec = np.zeros((C, NCV), np.float32)
    cvec[:, CV_BV] = b_v
    cvec[:, CV_W0] = w0v
    cvec[:, CV_BR1] = bgate1["r"] + gh1[0:C]
    cvec[:, CV_NBZ1] = -(bgate1["z"] + gh1[C:2 * C])
    cvec[:, CV_BN1] = bgate1["n"]
    cvec[:, CV_BR2] = b_ih[0:C] + b_hh[0:C]
    cvec[:, CV_NBZ2] = -(b_ih[C:2 * C] + b_hh[C:2 * C])
    cvec[:, CV_BIHN] = b_ih[2 * C:3 * C]
    cvec[:, CV_BHHN] = b_hh[2 * C:3 * C]
    cvec[:, CV_BP] = proj_b
    return wmat, cvec


def _run(inputs, trace=False):
    from concourse.bass_utils import run_bass_kernel_spmd

    x = np.asarray(inputs["x"], np.float32).reshape(B, C)
    wmat, cvec = _prep_consts(
        inputs["w0"], inputs["kv_w"], inputs["kv_b"], inputs["w_ih"],
        inputs["w_hh"], inputs["b_ih"], inputs["b_hh"], inputs["proj_w"],
        inputs["proj_b"])

    xT = _round_f32r(x.T)  # (C, B), f32r-rounded bits
    in_maps = []
    for c in range(NCORES):
        in_maps.append({
            "xT": np.ascontiguousarray(xT[:, c * BC:(c + 1) * BC]),
            "wmat": wmat,
            "cvec": cvec,
        })

    nc = _get_nc()
    res = run_bass_kernel_spmd(
        nc, in_maps, core_ids=list(range(NCORES)), trace=trace)
    outT = np.concatenate([res.results[c]["outT"] for c in range(NCORES)],
                          axis=1)  # (C, B)
    out = np.ascontiguousarray(outT.T).astype(np.float32)  # (B, C)
    return out, res


def kernel(**inputs):
    out, _ = _run(inputs, trace=False)
    return out
